# revision 5
# baseline (speedup 1.0000x reference)
"""Trainium2 Bass kernel for nn_CombineNode_7395933684091 (gnn_message_passing).

Hierarchy: 128 leaf terms (each D=1024 -> H=32), 16 internal terms
(concat of 8 children hiddens, 256 -> 32), 1 root (concat of 16
internal hiddens, 512 -> 32); every term also has a 1-dim predict head.
All matmuls followed by tanh.

Strategy: data-parallel over batch across 8 cores (Bc = 1024 rows per
core), weights replicated. On-chip layout keeps hidden features on the
PARTITION axis ("h^T layout": tiles are [features, batch]), so every
level's contraction is a natural PE matmul and the child-concat is just
stacking partition tiles. x and all weights are repacked on the host so
that every DMA is contiguous per partition:

  xt   [1024, 1024]  x[core].T                  (d on partitions)
  lw   [1024, 4096]  leaf_W.transpose(1,0,2)    row d, col l*32+h
  wpbd [128, 128]    block-diag leaf predict weights (4 leaves/group)
  intw [128, 1024]   int_W k-chunk j: row (c*32+h), col j*512+i*32+h'
  ...

Leaf level runs as 4 panels x 8 groups x 8 k-chunk accumulated matmuls
of [128,128] x [128,512]; predict heads use small block-diagonal
stationary operands accumulating into shared PSUM banks.

Matmuls stream as float32r (full-rate fp32 PE mode; plain float32 runs
4x slower). Set MM_DT = "float32" for the exact-precision path.
"""

import numpy as np

B, D, H = 8192, 1024, 32
L, I, CPI = 128, 16, 8
NCORES = 8
BC = B // NCORES      # 1024 batch rows per core
BN = 512              # batch tile width (one PSUM bank of f32)
NBH = BC // BN        # 2 batch halves
KC = D // 128         # 8 contraction chunks for the leaf level
NPANEL = 4            # leaf panels (8 groups of 4 leaves each)
GPP = 8               # groups per panel
NOUT = L + I + 1      # 145

MM_DT = "float32"

_CACHE = {}


def _build_nc():
    from contextlib import ExitStack

    import concourse.bass as bass
    import concourse.mybir as mybir
    import concourse.tile as tile
    from concourse import bacc

    f32 = mybir.dt.float32
    Tanh = mybir.ActivationFunctionType.Tanh
    mmdt = getattr(mybir.dt, MM_DT)

    nc = bacc.Bacc("TRN2", target_bir_lowering=False, debug=False)

    xt = nc.dram_tensor("xt", [D, BC], f32, kind="ExternalInput")
    lw = nc.dram_tensor("lw", [D, L * H], f32, kind="ExternalInput")
    lb = nc.dram_tensor("lb", [128, 32], f32, kind="ExternalInput")
    lbp = nc.dram_tensor("lbp", [32, 4], f32, kind="ExternalInput")
    wpbd = nc.dram_tensor("wpbd", [128, 32 * 32], f32, kind="ExternalInput")
    intw = nc.dram_tensor("intw", [128, 1024], f32, kind="ExternalInput")
    intb = nc.dram_tensor("intb", [128, 4], f32, kind="ExternalInput")
    intpbd = nc.dram_tensor("intpbd", [128, 64], f32, kind="ExternalInput")
    intbp = nc.dram_tensor("intbp", [16, 1], f32, kind="ExternalInput")
    rootw = nc.dram_tensor("rootw", [128, 128], f32, kind="ExternalInput")
    rootb = nc.dram_tensor("rootb", [32, 1], f32, kind="ExternalInput")
    rootwp = nc.dram_tensor("rootwp", [32, 1], f32, kind="ExternalInput")
    rootbp = nc.dram_tensor("rootbp", [1, 1], f32, kind="ExternalInput")
    out = nc.dram_tensor("out", [NOUT, BC], f32, kind="ExternalOutput")

    def mm(o, lhsT, rhs, **kw):
        nc.tensor.matmul(o, lhsT.bitcast(mmdt), rhs.bitcast(mmdt), **kw)

    with tile.TileContext(nc) as tc, ExitStack() as ctx:
        consts = ctx.enter_context(tc.tile_pool(name="consts", bufs=1))
        wpool = ctx.enter_context(tc.tile_pool(name="wpool", bufs=2))
        work = ctx.enter_context(tc.tile_pool(name="work", bufs=3))
        keep = ctx.enter_context(tc.tile_pool(name="keep", bufs=1))
        psum = ctx.enter_context(tc.tile_pool(name="psum", bufs=8, space="PSUM"))

        # --- resident loads -------------------------------------------------
        xt_sb = consts.tile([128, KC * BC], f32, name="xt_sb")
        for k in range(KC):
            nc.sync.dma_start(xt_sb[:, k * BC:(k + 1) * BC], xt[k * 128:(k + 1) * 128, :])
        lb_sb = consts.tile([128, 32], f32, name="lb_sb")
        nc.sync.dma_start(lb_sb[:], lb[:])
        lbp_sb = consts.tile([32, 4], f32, name="lbp_sb")
        nc.sync.dma_start(lbp_sb[:], lbp[:])
        wpbd_sb = consts.tile([128, 32 * 32], f32, name="wpbd_sb")
        nc.sync.dma_start(wpbd_sb[:], wpbd[:])
        intw_sb = consts.tile([128, 1024], f32, name="intw_sb")
        nc.sync.dma_start(intw_sb[:], intw[:])
        intb_sb = consts.tile([128, 4], f32, name="intb_sb")
        nc.sync.dma_start(intb_sb[:], intb[:])
        intpbd_sb = consts.tile([128, 64], f32, name="intpbd_sb")
        nc.sync.dma_start(intpbd_sb[:], intpbd[:])
        intbp_sb = consts.tile([16, 1], f32, name="intbp_sb")
        nc.sync.dma_start(intbp_sb[:], intbp[:])
        rootw_sb = consts.tile([128, 128], f32, name="rootw_sb")
        nc.sync.dma_start(rootw_sb[:], rootw[:])
        rootb_sb = consts.tile([32, 1], f32, name="rootb_sb")
        nc.sync.dma_start(rootb_sb[:], rootb[:])
        rootwp_sb = consts.tile([32, 1], f32, name="rootwp_sb")
        nc.sync.dma_start(rootwp_sb[:], rootwp[:])
        rootbp_sb = consts.tile([1, 1], f32, name="rootbp_sb")
        nc.sync.dma_start(rootbp_sb[:], rootbp[:])

        leafp_sb = keep.tile([128, BC], f32, name="leafp_sb")
        intp_sb = keep.tile([16, BC], f32, name="intp_sb")
        rootp_sb = keep.tile([1, BC], f32, name="rootp_sb")

        inth = {}  # (panel, bn) -> [128, BN] tile: int nodes 4p..4p+3 h^T

        # --- leaf + internal levels ----------------------------------------
        for p in range(NPANEL):
            wp = wpool.tile([128, KC * 1024], f32, tag="wpanel", name=f"wp{p}")
            for k in range(KC):
                nc.sync.dma_start(
                    wp[:, k * 1024:(k + 1) * 1024],
                    lw[k * 128:(k + 1) * 128, p * 1024:(p + 1) * 1024],
                )
            for bn in range(NBH):
                # two [64,BN] halves: matmul psum writes only at offsets 0/32/64
                pintA = psum.tile([64, BN], f32, tag="ps", name=f"pintA{p}{bn}")
                pintB = psum.tile([64, BN], f32, tag="ps", name=f"pintB{p}{bn}")
                plp = psum.tile([32, BN], f32, tag="ps", name=f"plp{p}{bn}")
                for il in range(4):
                    i = 4 * p + il
                    for j in range(2):
                        gl = 2 * il + j      # group in panel
                        g = GPP * p + gl     # global group (4 leaves)
                        pg = psum.tile([128, BN], f32, tag="ps", name=f"pg{p}{bn}{gl}")
                        for k in range(KC):
                            mm(
                                pg[:],
                                wp[:, k * 1024 + gl * 128:k * 1024 + (gl + 1) * 128],
                                xt_sb[:, k * BC + bn * BN:k * BC + bn * BN + BN],
                                start=(k == 0),
                                stop=(k == KC - 1),
                            )
                        lh = work.tile([128, BN], f32, tag="lh", name=f"lh{p}{bn}{gl}")
                        nc.scalar.activation(lh[:], pg[:], Tanh, bias=lb_sb[:, g:g + 1])
                        # leaf predict: block-diag [128,32] -> panel's 32
                        # leaf rows (zero cols outside this group's 4)
                        mm(
                            plp[:],
                            wpbd_sb[:, 32 * g:32 * g + 32],
                            lh[:],
                            start=(gl == 0),
                            stop=(gl == GPP - 1),
                            skip_group_check=True,
                        )
                        # internal trans: chunk j of node i
                        pint = pintA if il < 2 else pintB
                        mm(
                            pint[32 * (il % 2):32 * (il % 2) + 32, :],
                            intw_sb[:, j * 512 + 32 * i:j * 512 + 32 * i + 32],
                            lh[:],
                            start=(j == 0),
                            stop=(j == 1),
                            skip_group_check=True,
                        )
                ith = keep.tile([128, BN], f32, tag=f"inth{p}{bn}", name=f"inth{p}{bn}")
                nc.scalar.activation(ith[0:64, :], pintA[:], Tanh, bias=intb_sb[0:64, p:p + 1])
                nc.scalar.activation(ith[64:128, :], pintB[:], Tanh, bias=intb_sb[64:128, p:p + 1])
                inth[(p, bn)] = ith
                nc.scalar.activation(
                    leafp_sb[32 * p:32 * p + 32, bn * BN:bn * BN + BN],
                    plp[:],
                    Tanh,
                    bias=lbp_sb[:, p:p + 1],
                )

        # --- internal predict + root ---------------------------------------
        for bn in range(NBH):
            pip = psum.tile([16, BN], f32, tag="ps", name=f"pip{bn}")
            for p in range(NPANEL):
                mm(
                    pip[:],
                    intpbd_sb[:, 16 * p:16 * p + 16],
                    inth[(p, bn)][:],
                    start=(p == 0),
                    stop=(p == NPANEL - 1),
                    skip_group_check=True,
                )
            nc.scalar.activation(
                intp_sb[:, bn * BN:bn * BN + BN], pip[:], Tanh, bias=intbp_sb[:, 0:1]
            )
            prh = psum.tile([32, BN], f32, tag="ps", name=f"prh{bn}")
            for p in range(NPANEL):
                mm(
                    prh[:],
                    rootw_sb[:, 32 * p:32 * p + 32],
                    inth[(p, bn)][:],
                    start=(p == 0),
                    stop=(p == NPANEL - 1),
                    skip_group_check=True,
                )
            rh = work.tile([32, BN], f32, tag="rh", name=f"rh{bn}")
            nc.scalar.activation(rh[:], prh[:], Tanh, bias=rootb_sb[:, 0:1])
            prp = psum.tile([1, BN], f32, tag="ps", name=f"prp{bn}")
            mm(prp[:], rootwp_sb[:], rh[:], start=True, stop=True)
            nc.scalar.activation(
                rootp_sb[0:1, bn * BN:bn * BN + BN], prp[:], Tanh, bias=rootbp_sb[:, 0:1]
            )

        # --- stores ---------------------------------------------------------
        nc.sync.dma_start(out[0:L, :], leafp_sb[:])
        nc.sync.dma_start(out[L:L + I, :], intp_sb[:])
        nc.sync.dma_start(out[L + I:NOUT, :], rootp_sb[:])

    nc.compile()
    return nc


def _pack_weights(inp):
    f = np.float32
    leaf_W = np.asarray(inp["leaf_W"], f)
    leaf_b = np.asarray(inp["leaf_b"], f)
    int_W = np.asarray(inp["int_W"], f)
    int_b = np.asarray(inp["int_b"], f)
    root_W = np.asarray(inp["root_W"], f)
    root_b = np.asarray(inp["root_b"], f)
    leaf_Wp = np.asarray(inp["leaf_Wp"], f)
    leaf_bp = np.asarray(inp["leaf_bp"], f)
    int_Wp = np.asarray(inp["int_Wp"], f)
    int_bp = np.asarray(inp["int_bp"], f)
    root_Wp = np.asarray(inp["root_Wp"], f)
    root_bp = np.asarray(inp["root_bp"], f)

    w = {}
    w["lw"] = np.ascontiguousarray(leaf_W.transpose(1, 0, 2).reshape(D, L * H))
    w["lb"] = np.ascontiguousarray(leaf_b.reshape(32, 128).T)
    w["lbp"] = np.ascontiguousarray(leaf_bp.reshape(4, 32).T)
    wpbd = np.zeros((128, 32 * 32), f)
    for lv in range(L):
        g, c = divmod(lv, 4)       # group, leaf-in-group
        gl = g % GPP               # group within panel
        wpbd[c * 32:(c + 1) * 32, 32 * g + 4 * gl + c] = leaf_Wp[lv, :, 0]
    w["wpbd"] = wpbd
    w["intw"] = np.ascontiguousarray(
        np.concatenate(
            [
                int_W[:, 128 * j:128 * (j + 1), :].transpose(1, 0, 2).reshape(128, I * H)
                for j in range(2)
            ],
            axis=1,
        )
    )
    w["intb"] = np.ascontiguousarray(int_b.reshape(4, 128).T)
    intpbd = np.zeros((128, 64), f)
    for iv in range(I):
        p, c = divmod(iv, 4)       # panel, node-in-panel
        intpbd[c * 32:(c + 1) * 32, 16 * p + 4 * p + c] = int_Wp[iv, :, 0]
    w["intpbd"] = intpbd
    w["intbp"] = np.ascontiguousarray(int_bp.reshape(16, 1))
    w["rootw"] = np.ascontiguousarray(
        root_W.reshape(4, 128, 32).transpose(1, 0, 2).reshape(128, 128)
    )
    w["rootb"] = np.ascontiguousarray(root_b.reshape(32, 1))
    w["rootwp"] = np.ascontiguousarray(root_Wp.reshape(32, 1))
    w["rootbp"] = np.ascontiguousarray(root_bp.reshape(1, 1))
    return w


def kernel(**inputs):
    from concourse.bass_utils import run_bass_kernel_spmd

    nc = _CACHE.get("nc")
    if nc is None:
        nc = _CACHE["nc"] = _build_nc()

    x = np.asarray(inputs["x"], np.float32)
    w = _pack_weights(inputs)
    in_maps = []
    for c in range(NCORES):
        m = dict(w)
        m["xt"] = np.ascontiguousarray(x[c * BC:(c + 1) * BC, :].T)
        in_maps.append(m)

    res = run_bass_kernel_spmd(nc, in_maps, core_ids=list(range(NCORES)))
    _CACHE["last_res"] = res
    outs = [res.results[c]["out"] for c in range(NCORES)]
    full = np.concatenate([o[:, :, None] for o in outs], axis=1)  # [145, B, 1]
    return full.astype(np.float32)
